# revision 9
# baseline (speedup 1.0000x reference)
"""Trainium2 Bass kernel for the DecoderCRF loss (B=64, S=512, D=512, T=12).

Math
----
reference loss = sum_b [ logZ_b - gold_b ] with feats = x @ W.T + b.

For the transitions matrix this problem ships (row START == -1e4, col
STOP == -1e4, everything else 0) and an all-ones mask, the forward
recursion collapses exactly (verified vs a float64 port of the reference):

    logZ_b  = sum_t log( sum_{j=0..9} exp(feats[b,t,j]) )
    gold_b  = sum_t feats[b,t,tags[b,t]]

Layout strategy (v3)
--------------------
Data-parallel over batch: core c handles batch elements 8c..8c+7, one
512-step sequence per "group".  x ships as fp8 (e4m3) to halve DMA
bytes -- feats error ~0.05 abs on a ~N(0,1) distribution, loss rel err
~5e-4, far inside the 2e-2 gate.  (fp8 DoubleRow was measured to give
no real col/cycle gain on HW and its LDWEIGHTS fails the walrus ISA
check when combined with tile_position, so plain fp8 matmuls.)

Per core (xT fp8 [512 d, 4096 s], 4 panels of 1024 s-cols):
  1. feats^T: per panel-half (g = batch elem) 4 k-chunk matmuls into
     PSUM at partition offset 32*(g%4) via tile_position -- 4 groups
     stacked per PSUM bank: [128, 512] bank A (groups 0-3), B (4-7).
     W ships pre-scaled by 16 so its fp8 stays in the normal range.
  2. exp over the full [128, 512] bank on ScalarE (128 lanes, one op
     per bank) with bias=b and scale=1/16 folded in.
  3. colsum (LSE denominator): ONE matmul per bank with a [128, 4]
     selector -> psum rows 0-3; gold: DVE multiplies the feats bank by
     the stacked onehot, ONE ones-vector matmul -> psum row 32.
     (tensor_tensor_reduce would fuse the gold reduce on DVE but it
     crashes the device -- NRT_EXEC_UNIT_UNRECOVERABLE.)
  4. [33, 512] copy -> SBUF -> DMA out per bank.
Host sums log(colsums), row32/16 and the sum_t b[tag] term in f64.

The four xt panel DMAs are the FIRST instructions on gpsimd so SWDGE
descriptor generation (994ns fixed cost per call) starts immediately;
PE warmup matmuls (DVFS ramp + PSUM gap zeroing for the stacked banks,
which must be 0.0 not stale) overlap the stream.

Non-conforming inputs (different transitions pattern / mask / tag range)
fall back to a faithful numpy port of the reference.
"""

from contextlib import ExitStack

import numpy as np

N_CORES = 8
B, S, D = 64, 512, 512
T = 12
NT = 10          # tags that can actually appear / participate in the LSE
START, STOP = 10, 11
NEG = -10000.0
BS = B // N_CORES          # batch elements per core
R = BS * S                 # s-rows per core (4096)
N_PANELS = 4
PANEL = R // N_PANELS      # 1024
GROUP = 512                # s-cols per batch element
WSCALE = 16.0              # host pre-scale on W to keep fp8 normal

_NC_CACHE = None


def _build_nc():
    import concourse.bacc as bacc
    import concourse.mybir as mybir
    import concourse.tile as tile

    f32 = mybir.dt.float32
    bf16 = mybir.dt.bfloat16
    f8 = mybir.dt.float8e4
    nc = bacc.Bacc("TRN2", target_bir_lowering=False, num_swdge_queues=4)

    # Two superpanels; each partition's run = 4 d-chunks x 2048 s-cols of
    # xT (8KB) + that bank's 512B onehot row piggybacked -> one fat 8.7KB
    # descriptor per partition per superpanel (descriptor-overhead-bound
    # stream at 4KB runs measured only ~270GB/s).
    SPB = 4 * 2048 + GROUP          # 8704 bytes per partition per superpanel
    xt_d = nc.dram_tensor("xt", [2, 128, SPB], f8, kind="ExternalInput")
    wt_d = nc.dram_tensor("wt", [128, 4, NT], f8, kind="ExternalInput")
    sel_d = nc.dram_tensor("sel", [128, 4], bf16, kind="ExternalInput")
    b128_d = nc.dram_tensor("b128", [128, 1], f32, kind="ExternalInput")
    out_d = nc.dram_tensor("out", [2, 5, GROUP], f32, kind="ExternalOutput")

    with tile.TileContext(nc) as tc, ExitStack() as ctx:
        consts = ctx.enter_context(tc.tile_pool(name="consts", bufs=1))
        xtp = ctx.enter_context(tc.tile_pool(name="xtp", bufs=4))
        work = ctx.enter_context(tc.tile_pool(name="work", bufs=1))
        pfeat = ctx.enter_context(tc.tile_pool(name="pfeat", bufs=1, space="PSUM"))
        pout = ctx.enter_context(tc.tile_pool(name="pout", bufs=1, space="PSUM"))
        pwrm = ctx.enter_context(tc.tile_pool(name="pwrm", bufs=1, space="PSUM"))

        # ---- bulk stream first: SWDGE descriptor-gen starts immediately.
        # Four concurrent SWDGE calls (64-partition halves) -- aggregate
        # stream BW scales with in-flight calls, not descriptor size.
        sp_t = []
        for p in range(2):
            t = xtp.tile([128, SPB], f8, tag=f"sp{p}", name=f"sp{p}")
            sp_t.append(t)
        for p in range(2):
            for half in range(2):
                lo = 64 * half
                nc.gpsimd.dma_start(out=sp_t[p][lo : lo + 64, :],
                                    in_=xt_d[p, lo : lo + 64, :])

        # ---- small consts on the two HWDGE queues (SP + Activation)
        wt_sb = consts.tile([128, 4, NT], f8)
        nc.sync.dma_start(out=wt_sb, in_=wt_d[:, :, :])
        b128_sb = consts.tile([128, 1], f32)
        nc.sync.dma_start(out=b128_sb, in_=b128_d[:, :])
        sel_sb = consts.tile([128, 4], bf16)
        nc.scalar.dma_start(out=sel_sb, in_=sel_d[:, :])

        wz = work.tile([128, GROUP], bf16, tag="wz")
        nc.vector.memset(wz, 0.0)
        ones128 = work.tile([128, 1], bf16, tag="ones")
        nc.vector.memset(ones128, 1.0)

        # preload the exp table on ScalarE while the stream runs
        dummy = work.tile([128, 1], bf16, tag="dummy")
        nc.scalar.activation(dummy, wz[:, 0:1], mybir.ActivationFunctionType.Exp)

        # ---- PE warmup: DVFS ramp + zero the stacked feats banks
        # (the partition gaps between groups must be 0.0, not stale PSUM)
        psum_w = pwrm.tile([128, GROUP], f32, tag="warm")
        feats = [
            pfeat.tile([128, GROUP], f32, tag="fA", name="fA"),
            pfeat.tile([128, GROUP], f32, tag="fB", name="fB"),
        ]
        for _ in range(6):
            nc.tensor.matmul(psum_w, lhsT=wz[:, 0:128], rhs=wz, start=True, stop=True)
        for bk in range(2):
            nc.tensor.matmul(feats[bk], lhsT=wz[:, 0:128], rhs=wz,
                             start=True, stop=True)

        outs = [pout.tile([33, GROUP], f32, tag=f"o{bk}", name=f"o{bk}")
                for bk in range(2)]
        e_sb = [work.tile([128, GROUP], bf16, tag=f"e{bk}", name=f"e{bk}")
                for bk in range(2)]
        osb = [work.tile([33, GROUP], f32, tag=f"ob{bk}", name=f"ob{bk}")
               for bk in range(2)]
        gw_scr = work.tile([128, GROUP], bf16, tag="gwscr")
        oht_of = 4 * 2048

        def bank_head(bk):
            # as soon as feats bank bk is complete: exp + fused gold reduce
            nc.scalar.activation(
                e_sb[bk], feats[bk], mybir.ActivationFunctionType.Exp,
                bias=b128_sb[:, :], scale=1.0 / WSCALE,
            )
            nc.vector.tensor_mul(gw_scr, feats[bk],
                                 sp_t[bk][:, oht_of : oht_of + GROUP])

        def bank_pe(bk):
            nc.tensor.matmul(outs[bk][0:4, :], lhsT=sel_sb, rhs=e_sb[bk],
                             start=True, stop=True)
            nc.tensor.matmul(outs[bk][32:33, :], lhsT=ones128, rhs=gw_scr,
                             start=True, stop=True, tile_position=(0, 32))
            nc.scalar.copy(out=osb[bk], in_=outs[bk])
            nc.sync.dma_start(out=out_d[bk, 0:4], in_=osb[bk][0:4, :])
            nc.sync.dma_start(out=out_d[bk, 4:5], in_=osb[bk][32:33, :])

        for sp in range(2):
            bank = feats[sp]
            for h in range(4):
                q = 32 * h
                for dc in range(4):
                    a = dc * 2048 + h * GROUP
                    nc.tensor.matmul(
                        bank[q : q + NT, :],
                        lhsT=wt_sb[:, dc, :],
                        rhs=sp_t[sp][:, a : a + GROUP],
                        start=(dc == 0),
                        stop=(dc == 3),
                        tile_position=(0, q),
                    )
            bank_head(sp)
            if sp == 0:
                bank_pe(0)
        bank_pe(1)

    nc.compile()
    return nc


def _get_nc():
    global _NC_CACHE
    if _NC_CACHE is None:
        _NC_CACHE = _build_nc()
    return _NC_CACHE


def _fast_path_ok(transitions, tags, mask):
    if transitions.shape != (T, T) or tags.min() < 0 or tags.max() >= NT:
        return False
    if not np.all(mask == 1):
        return False
    t2 = np.asarray(transitions, np.float64).copy()
    if not (np.all(t2[START, :] == NEG) and np.all(t2[:, STOP] == NEG)):
        return False
    t2[START, :] = 0.0
    t2[:, STOP] = 0.0
    return bool(np.all(t2 == 0.0))


def _reference_numpy(input_var, W, b, transitions, tags, mask):
    """Faithful float64 port of the reference (fallback only)."""
    x = np.asarray(input_var, np.float64)
    Wf = np.asarray(W, np.float64)
    bf = np.asarray(b, np.float64)
    tr = np.asarray(transitions, np.float64)
    mf = np.asarray(mask, np.float64)
    Bn, Sn, Dn = x.shape
    feats = (x.reshape(-1, Dn) @ Wf.T + bf).reshape(Bn, Sn, -1)
    fv = np.full((Bn, T), NEG)
    fv[:, START] = 0.0
    for t in range(Sn):
        tv = fv[:, None, :] + tr[None] + feats[:, t][:, :, None]
        m = tv.max(axis=2)
        new = m + np.log(np.exp(tv - m[:, :, None]).sum(axis=2))
        fv = new * mf[:, t : t + 1] + fv * (1 - mf[:, t : t + 1])
    fin = fv + tr[STOP][None]
    mm = fin.max(axis=1)
    alpha = mm + np.log(np.exp(fin - mm[:, None]).sum(axis=1))
    score0 = tr[tags[:, 0], START]
    emit = np.take_along_axis(feats[:, :-1], tags[:, :-1, None], axis=2)[..., 0]
    emit_sum = (emit * mf[:, :-1]).sum(axis=1)
    trs = tr[tags[:, 1:], tags[:, :-1]]
    trans_sum = (trs * mf[:, 1:]).sum(axis=1)
    last_idx = np.asarray(mask).sum(axis=1).astype(np.int64) - 1
    last_tags = np.take_along_axis(tags, last_idx[:, None], axis=1)[:, 0]
    last_emit = np.take_along_axis(feats[:, -1], last_tags[:, None], axis=1)[:, 0]
    gold = score0 + emit_sum + trans_sum + tr[STOP, last_tags] + last_emit * mf[:, -1]
    return np.float32((alpha - gold).sum())


def _make_in_maps(input_var, W, b, tags):
    import ml_dtypes

    f8 = ml_dtypes.float8_e4m3
    bf16 = ml_dtypes.bfloat16

    Wv = np.asarray(W, np.float32)
    # wt[part, dc, m] = WSCALE * W[m, dc*128+part]
    wt = np.ascontiguousarray(
        (WSCALE * Wv[:NT].T).reshape(4, 128, NT).transpose(1, 0, 2)
    ).astype(f8)
    SPB = 4 * 2048 + GROUP

    b128 = np.zeros((128, 1), np.float32)
    sel = np.zeros((128, 4), np.float32)
    for gq in range(4):
        b128[32 * gq : 32 * gq + NT, 0] = np.asarray(b, np.float32)[:NT]
        sel[32 * gq + np.arange(NT), gq] = 1.0
    sel = sel.astype(bf16)

    x8 = np.asarray(input_var).astype(f8)                      # one big cast
    tg = np.asarray(tags)

    in_maps = []
    for c in range(N_CORES):
        xc = x8[BS * c : BS * (c + 1)].reshape(R, D)           # [4096, 512]
        xt = np.empty((2, 128, SPB), f8)
        for sp in range(2):
            blk = xc[sp * 2048 : (sp + 1) * 2048]              # [2048, 512]
            # [s, dc, part] -> [part, dc, s] -> [part, 8192]
            xt[sp, :, : 4 * 2048] = np.ascontiguousarray(
                blk.reshape(2048, 4, 128).transpose(2, 1, 0)
            ).reshape(128, 4 * 2048)
            oht = np.zeros((128, GROUP), np.float32)
            for gq in range(4):
                row = tg[BS * c + 4 * sp + gq]                 # [512]
                oht[32 * gq : 32 * gq + NT, :] = (
                    row[None, :] == np.arange(NT)[:, None]
                )
            xt[sp, :, 4 * 2048 :] = oht.astype(f8)
        in_maps.append({"xt": xt, "wt": wt, "sel": sel, "b128": b128})
    return in_maps


def kernel(input_var, W, b, transitions, tags, mask):
    from concourse.bass_utils import run_bass_kernel_spmd

    input_var = np.asarray(input_var)
    W = np.asarray(W)
    b = np.asarray(b)
    transitions = np.asarray(transitions)
    tags = np.asarray(tags)
    mask = np.asarray(mask)

    if not _fast_path_ok(transitions, tags, mask):
        return _reference_numpy(input_var, W, b, transitions, tags, mask)

    nc = _get_nc()
    in_maps = _make_in_maps(input_var, W, b, tags)
    res = run_bass_kernel_spmd(nc, in_maps, list(range(N_CORES)))

    total = np.float64(0.0)
    for c in range(N_CORES):
        out = np.asarray(res.results[c]["out"], np.float64)    # [2, 5, 512]
        total += np.log(out[:, :4, :]).sum() - out[:, 4, :].sum() / WSCALE
    total -= np.asarray(b, np.float64)[tags].sum()             # gold bias term
    return np.float32(total)


# revision 10
# speedup vs baseline: 1.1226x; 1.1226x over previous
"""Trainium2 Bass kernel for the DecoderCRF loss (B=64, S=512, D=512, T=12).

Math
----
reference loss = sum_b [ logZ_b - gold_b ] with feats = x @ W.T + b.

For the transitions matrix this problem ships (row START == -1e4, col
STOP == -1e4, everything else 0) and an all-ones mask, the forward
recursion collapses exactly (verified vs a float64 port of the reference):

    logZ_b  = sum_t log( sum_{j=0..9} exp(feats[b,t,j]) )
    gold_b  = sum_t feats[b,t,tags[b,t]]

Layout strategy (v3)
--------------------
Data-parallel over batch: core c handles batch elements 8c..8c+7, one
512-step sequence per "group".  x ships as fp8 (e4m3) to halve DMA
bytes -- feats error ~0.05 abs on a ~N(0,1) distribution, loss rel err
~5e-4, far inside the 2e-2 gate.  (fp8 DoubleRow was measured to give
no real col/cycle gain on HW and its LDWEIGHTS fails the walrus ISA
check when combined with tile_position, so plain fp8 matmuls.)

Per core (xT fp8 [512 d, 4096 s], 4 panels of 1024 s-cols):
  1. feats^T: per panel-half (g = batch elem) 4 k-chunk matmuls into
     PSUM at partition offset 32*(g%4) via tile_position -- 4 groups
     stacked per PSUM bank: [128, 512] bank A (groups 0-3), B (4-7).
     W ships pre-scaled by 16 so its fp8 stays in the normal range.
  2. exp over the full [128, 512] bank on ScalarE (128 lanes, one op
     per bank) with bias=b and scale=1/16 folded in.
  3. colsum (LSE denominator): ONE matmul per bank with a [128, 4]
     selector -> psum rows 0-3; gold: DVE multiplies the feats bank by
     the stacked onehot, ONE ones-vector matmul -> psum row 32.
     (tensor_tensor_reduce would fuse the gold reduce on DVE but it
     crashes the device -- NRT_EXEC_UNIT_UNRECOVERABLE.)
  4. [33, 512] copy -> SBUF -> DMA out per bank.
Host sums log(colsums), row32/16 and the sum_t b[tag] term in f64.

The four xt panel DMAs are the FIRST instructions on gpsimd so SWDGE
descriptor generation (994ns fixed cost per call) starts immediately;
PE warmup matmuls (DVFS ramp + PSUM gap zeroing for the stacked banks,
which must be 0.0 not stale) overlap the stream.

Non-conforming inputs (different transitions pattern / mask / tag range)
fall back to a faithful numpy port of the reference.
"""

from contextlib import ExitStack

import numpy as np

N_CORES = 8
B, S, D = 64, 512, 512
T = 12
NT = 10          # tags that can actually appear / participate in the LSE
START, STOP = 10, 11
NEG = -10000.0
BS = B // N_CORES          # batch elements per core
R = BS * S                 # s-rows per core (4096)
N_PANELS = 4
PANEL = R // N_PANELS      # 1024
GROUP = 512                # s-cols per batch element
WSCALE = 16.0              # host pre-scale on W to keep fp8 normal

_NC_CACHE = None


def _build_nc():
    import concourse.bacc as bacc
    import concourse.mybir as mybir
    import concourse.tile as tile

    f32 = mybir.dt.float32
    bf16 = mybir.dt.bfloat16
    f8 = mybir.dt.float8e4
    nc = bacc.Bacc("TRN2", target_bir_lowering=False, num_swdge_queues=4)

    # Two superpanels; each partition's run = 4 d-chunks x 2048 s-cols of
    # xT (8KB) + that bank's 512B onehot row piggybacked -> one fat 8.7KB
    # descriptor per partition per superpanel (descriptor-overhead-bound
    # stream at 4KB runs measured only ~270GB/s).
    SPB = 4 * 2048 + GROUP          # 8704 bytes per partition per superpanel
    u32 = mybir.dt.uint32
    # DMA moves ~230G elements/s regardless of element size, so ship the
    # fp8 payload as uint32 words (4x the byte rate); compute reads the
    # same SBUF bytes through the fp8 tile APs.
    xt_d = nc.dram_tensor("xt", [2, 128, SPB // 4], u32, kind="ExternalInput")
    wt_d = nc.dram_tensor("wt", [128, 4, NT], f8, kind="ExternalInput")
    sel_d = nc.dram_tensor("sel", [128, 4], bf16, kind="ExternalInput")
    b128_d = nc.dram_tensor("b128", [128, 1], f32, kind="ExternalInput")
    out_d = nc.dram_tensor("out", [2, 5, GROUP], f32, kind="ExternalOutput")

    with tile.TileContext(nc) as tc, ExitStack() as ctx:
        consts = ctx.enter_context(tc.tile_pool(name="consts", bufs=1))
        xtp = ctx.enter_context(tc.tile_pool(name="xtp", bufs=4))
        work = ctx.enter_context(tc.tile_pool(name="work", bufs=1))
        pfeat = ctx.enter_context(tc.tile_pool(name="pfeat", bufs=1, space="PSUM"))
        pout = ctx.enter_context(tc.tile_pool(name="pout", bufs=1, space="PSUM"))
        pwrm = ctx.enter_context(tc.tile_pool(name="pwrm", bufs=1, space="PSUM"))

        # ---- bulk stream first: SWDGE descriptor-gen starts immediately
        sp_t = []
        for p in range(2):
            t = xtp.tile([128, SPB], f8, tag=f"sp{p}", name=f"sp{p}")
            sp_t.append(t)
            nc.gpsimd.dma_start(out=t.bitcast(u32), in_=xt_d[p])

        # ---- small consts on the two HWDGE queues (SP + Activation)
        wt_sb = consts.tile([128, 4, NT], f8)
        nc.sync.dma_start(out=wt_sb, in_=wt_d[:, :, :])
        b128_sb = consts.tile([128, 1], f32)
        nc.sync.dma_start(out=b128_sb, in_=b128_d[:, :])
        sel_sb = consts.tile([128, 4], bf16)
        nc.scalar.dma_start(out=sel_sb, in_=sel_d[:, :])

        wz = work.tile([128, GROUP], bf16, tag="wz")
        nc.vector.memset(wz, 0.0)
        ones128 = work.tile([128, 1], bf16, tag="ones")
        nc.vector.memset(ones128, 1.0)

        # preload the exp table on ScalarE while the stream runs
        dummy = work.tile([128, 1], bf16, tag="dummy")
        nc.scalar.activation(dummy, wz[:, 0:1], mybir.ActivationFunctionType.Exp)

        # ---- PE warmup: DVFS ramp + zero the stacked feats banks
        # (the partition gaps between groups must be 0.0, not stale PSUM)
        psum_w = pwrm.tile([128, GROUP], f32, tag="warm")
        feats = [
            pfeat.tile([128, GROUP], f32, tag="fA", name="fA"),
            pfeat.tile([128, GROUP], f32, tag="fB", name="fB"),
        ]
        for _ in range(6):
            nc.tensor.matmul(psum_w, lhsT=wz[:, 0:128], rhs=wz, start=True, stop=True)
        for bk in range(2):
            nc.tensor.matmul(feats[bk], lhsT=wz[:, 0:128], rhs=wz,
                             start=True, stop=True)

        outs = [pout.tile([33, GROUP], f32, tag=f"o{bk}", name=f"o{bk}")
                for bk in range(2)]
        e_sb = [work.tile([128, GROUP], bf16, tag=f"e{bk}", name=f"e{bk}")
                for bk in range(2)]
        osb = [work.tile([33, GROUP], f32, tag=f"ob{bk}", name=f"ob{bk}")
               for bk in range(2)]
        gw_scr = work.tile([128, GROUP], bf16, tag="gwscr")
        oht_of = 4 * 2048

        def bank_head(bk):
            # as soon as feats bank bk is complete: exp + fused gold reduce
            nc.scalar.activation(
                e_sb[bk], feats[bk], mybir.ActivationFunctionType.Exp,
                bias=b128_sb[:, :], scale=1.0 / WSCALE,
            )
            nc.vector.tensor_mul(gw_scr, feats[bk],
                                 sp_t[bk][:, oht_of : oht_of + GROUP])

        def bank_pe(bk):
            nc.tensor.matmul(outs[bk][0:4, :], lhsT=sel_sb, rhs=e_sb[bk],
                             start=True, stop=True)
            nc.tensor.matmul(outs[bk][32:33, :], lhsT=ones128, rhs=gw_scr,
                             start=True, stop=True, tile_position=(0, 32))
            nc.scalar.copy(out=osb[bk], in_=outs[bk])
            nc.sync.dma_start(out=out_d[bk, 0:4], in_=osb[bk][0:4, :])
            nc.sync.dma_start(out=out_d[bk, 4:5], in_=osb[bk][32:33, :])

        for sp in range(2):
            bank = feats[sp]
            for h in range(4):
                q = 32 * h
                for dc in range(4):
                    a = dc * 2048 + h * GROUP
                    nc.tensor.matmul(
                        bank[q : q + NT, :],
                        lhsT=wt_sb[:, dc, :],
                        rhs=sp_t[sp][:, a : a + GROUP],
                        start=(dc == 0),
                        stop=(dc == 3),
                        tile_position=(0, q),
                    )
            bank_head(sp)
            if sp == 0:
                bank_pe(0)
        bank_pe(1)

    nc.compile()
    return nc


def _get_nc():
    global _NC_CACHE
    if _NC_CACHE is None:
        _NC_CACHE = _build_nc()
    return _NC_CACHE


def _fast_path_ok(transitions, tags, mask):
    if transitions.shape != (T, T) or tags.min() < 0 or tags.max() >= NT:
        return False
    if not np.all(mask == 1):
        return False
    t2 = np.asarray(transitions, np.float64).copy()
    if not (np.all(t2[START, :] == NEG) and np.all(t2[:, STOP] == NEG)):
        return False
    t2[START, :] = 0.0
    t2[:, STOP] = 0.0
    return bool(np.all(t2 == 0.0))


def _reference_numpy(input_var, W, b, transitions, tags, mask):
    """Faithful float64 port of the reference (fallback only)."""
    x = np.asarray(input_var, np.float64)
    Wf = np.asarray(W, np.float64)
    bf = np.asarray(b, np.float64)
    tr = np.asarray(transitions, np.float64)
    mf = np.asarray(mask, np.float64)
    Bn, Sn, Dn = x.shape
    feats = (x.reshape(-1, Dn) @ Wf.T + bf).reshape(Bn, Sn, -1)
    fv = np.full((Bn, T), NEG)
    fv[:, START] = 0.0
    for t in range(Sn):
        tv = fv[:, None, :] + tr[None] + feats[:, t][:, :, None]
        m = tv.max(axis=2)
        new = m + np.log(np.exp(tv - m[:, :, None]).sum(axis=2))
        fv = new * mf[:, t : t + 1] + fv * (1 - mf[:, t : t + 1])
    fin = fv + tr[STOP][None]
    mm = fin.max(axis=1)
    alpha = mm + np.log(np.exp(fin - mm[:, None]).sum(axis=1))
    score0 = tr[tags[:, 0], START]
    emit = np.take_along_axis(feats[:, :-1], tags[:, :-1, None], axis=2)[..., 0]
    emit_sum = (emit * mf[:, :-1]).sum(axis=1)
    trs = tr[tags[:, 1:], tags[:, :-1]]
    trans_sum = (trs * mf[:, 1:]).sum(axis=1)
    last_idx = np.asarray(mask).sum(axis=1).astype(np.int64) - 1
    last_tags = np.take_along_axis(tags, last_idx[:, None], axis=1)[:, 0]
    last_emit = np.take_along_axis(feats[:, -1], last_tags[:, None], axis=1)[:, 0]
    gold = score0 + emit_sum + trans_sum + tr[STOP, last_tags] + last_emit * mf[:, -1]
    return np.float32((alpha - gold).sum())


def _make_in_maps(input_var, W, b, tags):
    import ml_dtypes

    f8 = ml_dtypes.float8_e4m3
    bf16 = ml_dtypes.bfloat16

    Wv = np.asarray(W, np.float32)
    # wt[part, dc, m] = WSCALE * W[m, dc*128+part]
    wt = np.ascontiguousarray(
        (WSCALE * Wv[:NT].T).reshape(4, 128, NT).transpose(1, 0, 2)
    ).astype(f8)
    SPB = 4 * 2048 + GROUP

    b128 = np.zeros((128, 1), np.float32)
    sel = np.zeros((128, 4), np.float32)
    for gq in range(4):
        b128[32 * gq : 32 * gq + NT, 0] = np.asarray(b, np.float32)[:NT]
        sel[32 * gq + np.arange(NT), gq] = 1.0
    sel = sel.astype(bf16)

    x8 = np.asarray(input_var).astype(f8)                      # one big cast
    tg = np.asarray(tags)

    in_maps = []
    for c in range(N_CORES):
        xc = x8[BS * c : BS * (c + 1)].reshape(R, D)           # [4096, 512]
        xt = np.empty((2, 128, SPB), f8)
        for sp in range(2):
            blk = xc[sp * 2048 : (sp + 1) * 2048]              # [2048, 512]
            # [s, dc, part] -> [part, dc, s] -> [part, 8192]
            xt[sp, :, : 4 * 2048] = np.ascontiguousarray(
                blk.reshape(2048, 4, 128).transpose(2, 1, 0)
            ).reshape(128, 4 * 2048)
            oht = np.zeros((128, GROUP), np.float32)
            for gq in range(4):
                row = tg[BS * c + 4 * sp + gq]                 # [512]
                oht[32 * gq : 32 * gq + NT, :] = (
                    row[None, :] == np.arange(NT)[:, None]
                )
            xt[sp, :, 4 * 2048 :] = oht.astype(f8)
        in_maps.append({"xt": xt.view(np.uint8).view(np.uint32), "wt": wt,
                        "sel": sel, "b128": b128})
    return in_maps


def kernel(input_var, W, b, transitions, tags, mask):
    from concourse.bass_utils import run_bass_kernel_spmd

    input_var = np.asarray(input_var)
    W = np.asarray(W)
    b = np.asarray(b)
    transitions = np.asarray(transitions)
    tags = np.asarray(tags)
    mask = np.asarray(mask)

    if not _fast_path_ok(transitions, tags, mask):
        return _reference_numpy(input_var, W, b, transitions, tags, mask)

    nc = _get_nc()
    in_maps = _make_in_maps(input_var, W, b, tags)
    res = run_bass_kernel_spmd(nc, in_maps, list(range(N_CORES)))

    total = np.float64(0.0)
    for c in range(N_CORES):
        out = np.asarray(res.results[c]["out"], np.float64)    # [2, 5, 512]
        total += np.log(out[:, :4, :]).sum() - out[:, 4, :].sum() / WSCALE
    total -= np.asarray(b, np.float64)[tags].sum()             # gold bias term
    return np.float32(total)
